# revision 1
# baseline (speedup 1.0000x reference)
"""Masked L1 loss (per-(b,c) normalized) on 8 Trainium2 NeuronCores.

Layout: batch-dim data parallel. Core i takes batches [2i, 2i+2) of the
[16, 64, 128, 128] inputs -> a [128, 16384] shard (partition = (b, c) pair,
free = h*w). Per [128, 2048] tile:
    DVE  tensor_tensor           sd = pre - gt
    ACT  activation(Abs)         ad = |sd|
    DVE  scalar_tensor_tensor    junk = ad * mask, accum l1_part[p] += sum
    DVE  tensor_reduce(add)      ct_part[p] = sum(mask)  (mask is 0/1)
DMA-bound by design (~24 MiB/core HBM reads; DVE 3 passes, ACT 1).
Per-(b,c) tile partials land in [128, NT] accumulators, DMA'd to DRAM.
Host: l1 = partials.sum, ct = partials.sum, loss = sum(l1/max(ct,1))/B.
"""

import sys

if "/opt/trn_rl_repo" not in sys.path:
    sys.path.insert(0, "/opt/trn_rl_repo")

import numpy as np

B, C, H, W = 16, 64, 128, 128
N_CORES = 8
BPC = B // N_CORES          # batches per core = 2
P = BPC * C                 # partitions per core = 128 (one (b,c) pair each)
HW = H * W                  # 16384 free elements per partition
T = 2048                    # free-dim tile size
NT = HW // T                # 8 tiles

_CACHE = {}


def _build(reps=1, t=T, io_bufs=3, work_bufs=3):
    key = ("nc", reps, t, io_bufs, work_bufs)
    if key in _CACHE:
        return _CACHE[key]
    nt = HW // t

    import contextlib

    import concourse.bacc as bacc
    import concourse.mybir as mybir
    from concourse.tile import TileContext

    f32 = mybir.dt.float32
    Alu = mybir.AluOpType
    Act = mybir.ActivationFunctionType

    nc = bacc.Bacc(
        "TRN2",
        target_bir_lowering=False,
        debug=False,
        enable_asserts=False,
        num_devices=N_CORES,
    )

    pre = nc.dram_tensor("pre", [P, HW], f32, kind="ExternalInput").ap()
    gt = nc.dram_tensor("gt", [P, HW], f32, kind="ExternalInput").ap()
    mask = nc.dram_tensor("mask", [P, HW], f32, kind="ExternalInput").ap()
    out = nc.dram_tensor("out", [P, 2 * nt], f32, kind="ExternalOutput").ap()

    with TileContext(nc) as tc:
        with (
            tc.tile_pool(name="io", bufs=io_bufs) as io,
            tc.tile_pool(name="work", bufs=work_bufs) as work,
            tc.tile_pool(name="acc", bufs=1) as accp,
        ):
            l1p = accp.tile([P, nt], f32, tag="l1p")
            ctp = accp.tile([P, nt], f32, tag="ctp")

            # reps>1 is a benchmarking amplifier: repeat the identical full
            # pass inside one NEFF so per-pass time is resolvable above the
            # per-call RPC noise. The last pass's results win (all identical).
            rep_ctx = tc.For_i(0, reps, 1) if reps > 1 else contextlib.nullcontext()
            with rep_ctx:
                for i in range(nt):
                    tp = io.tile([P, t], f32, tag="pre")
                    tg = io.tile([P, t], f32, tag="gt")
                    tm = io.tile([P, t], f32, tag="mask")
                    nc.sync.dma_start(out=tp, in_=pre[:, i * t : (i + 1) * t])
                    nc.sync.dma_start(out=tg, in_=gt[:, i * t : (i + 1) * t])
                    nc.sync.dma_start(out=tm, in_=mask[:, i * t : (i + 1) * t])

                    sd = work.tile([P, t], f32, tag="sd")
                    ad = work.tile([P, t], f32, tag="ad")

                    nc.vector.tensor_tensor(out=sd, in0=tp, in1=tg, op=Alu.subtract)
                    nc.scalar.activation(out=ad, in_=sd, func=Act.Abs)
                    # one DVE pass: junk = ad * mask, l1 partial = sum(junk)
                    nc.vector.scalar_tensor_tensor(
                        out=sd,
                        in0=ad,
                        scalar=0.0,
                        in1=tm,
                        op0=Alu.bypass,
                        op1=Alu.mult,
                        accum_out=l1p[:, i : i + 1],
                    )
                    # mask is 0/1 so sum(mask) == nonzero count
                    nc.vector.tensor_reduce(
                        out=ctp[:, i : i + 1],
                        in_=tm,
                        axis=mybir.AxisListType.X,
                        op=Alu.add,
                    )

            nc.sync.dma_start(out=out[:, 0:nt], in_=l1p)
            nc.sync.dma_start(out=out[:, nt : 2 * nt], in_=ctp)

    nc.compile()
    _CACHE[key] = nc
    return nc


def _shard(pre, gt, mask):
    in_maps = []
    for i in range(N_CORES):
        sl = slice(i * BPC, (i + 1) * BPC)
        in_maps.append(
            {
                "pre": np.ascontiguousarray(pre[sl], dtype=np.float32).reshape(P, HW),
                "gt": np.ascontiguousarray(gt[sl], dtype=np.float32).reshape(P, HW),
                "mask": np.ascontiguousarray(mask[sl], dtype=np.float32).reshape(P, HW),
            }
        )
    return in_maps


def _combine(results, batch_size):
    total = np.float32(0.0)
    for r in results:
        o = np.asarray(r["out"], dtype=np.float32)
        nt = o.shape[1] // 2
        l1 = o[:, :nt].sum(axis=1, dtype=np.float32)
        ct = o[:, nt:].sum(axis=1, dtype=np.float32)
        total += (l1 / np.maximum(ct, np.float32(1.0))).sum(dtype=np.float32)
    return np.asarray(total / np.float32(int(batch_size)), dtype=np.float32)


def run(pre, gt, mask, batch_size, trace=False, **bass_kwargs):
    from concourse.bass_utils import run_bass_kernel_spmd

    nc = _build()
    in_maps = _shard(np.asarray(pre), np.asarray(gt), np.asarray(mask))
    res = run_bass_kernel_spmd(
        nc, in_maps, list(range(N_CORES)), trace=trace, **bass_kwargs
    )
    loss = _combine(res.results, batch_size)
    return loss, res


def kernel(pre, gt, mask, batch_size):
    loss, _ = run(pre, gt, mask, batch_size)
    return loss



# revision 22
# speedup vs baseline: 1.0416x; 1.0416x over previous
"""Masked L1 loss (per-(b,c) normalized) on 8 Trainium2 NeuronCores.

Layout: batch-dim data parallel. Core i takes batches [2i, 2i+2) of the
[16, 64, 128, 128] inputs -> a [128, 16384] shard (partition = (b, c) pair,
free = h*w). Per [128, t] tile (t tapers 2048 -> 256 at the end to shorten
the post-last-DMA tail):
    DVE  tensor_tensor          sd = pre - gt
    DVE  tensor_tensor          dm = sd * mask
    ACT  activation(Copy)+accum ct_col = sum(mask)   (mask is 0/1)
    DVE  scalar_tensor_tensor   junk = (dm * -1) max dm = |dm|,
                                accum l1_col = sum   (all ISA-valid ALU ops;
                                mask is 0/1 so |dm| = |pre-gt|*mask)
DMA-bound by design (~24 MiB/core HBM reads; DVE 3 passes, ACT 1 pass,
both under the DMA stream time, so they hide; the whole l1 chain stays on
DVE so the post-last-DMA tail has no cross-engine hops).
Per-(b,c) tile partials land in one [128, 2*NT] accumulator (l1 in even
columns, count in odd), DMA'd to DRAM once at the end.
Host: l1 = even.sum, ct = odd.sum, loss = sum(l1/max(ct,1))/B.
"""

import sys

if "/opt/trn_rl_repo" not in sys.path:
    sys.path.insert(0, "/opt/trn_rl_repo")

import numpy as np

B, C, H, W = 16, 64, 128, 128
N_CORES = 8
BPC = B // N_CORES          # batches per core = 2
P = BPC * C                 # partitions per core = 128 (one (b,c) pair each)
HW = H * W                  # 16384 free elements per partition
T = 2048                    # max free-dim tile size
# Taper the end of the schedule: the last tiles are small so the critical
# path after the final input DMA (sem + stt + out-DMA) is short.
TILE_SIZES = [2048] * 7 + [1024, 704, 320]
# per-tile l1-chain placement: "dve" = |dm| sum via DVE stt (3rd DVE pass),
# "act" = |dm| sum via ACT activation(Abs)+accum (2 DVE passes + 1 ACT pass).
# Body tiles on ACT keep DVE at 2 passes/tile (no backlog at the taper);
# the last tile's chain stays entirely on DVE so the tail has no engine hop.
TILE_MODES = ["act"] * 9 + ["dve"]
# count placement is ACT everywhere: the Pool engine's ISA rejects
# TENSOR_SCALAR_PTR (walrus neuron_isa_check_opcode_on_engine), so the
# count cannot ride gpsimd
COUNT_MODES = ["act"] * 10
IO_BUFS = 3
WORK_BUFS = 3
assert sum(TILE_SIZES) == HW
NT = len(TILE_SIZES)

_CACHE = {}


def _build():
    key = ("nc", tuple(TILE_SIZES), tuple(TILE_MODES), tuple(COUNT_MODES),
           T, IO_BUFS, WORK_BUFS)
    if key in _CACHE:
        return _CACHE[key]

    import concourse.bacc as bacc
    import concourse.mybir as mybir
    from concourse.tile import TileContext

    f32 = mybir.dt.float32
    Alu = mybir.AluOpType
    Act = mybir.ActivationFunctionType

    nc = bacc.Bacc(
        "TRN2",
        target_bir_lowering=False,
        debug=False,
        enable_asserts=False,
        num_devices=N_CORES,
    )

    pre = nc.dram_tensor("pre", [P, HW], f32, kind="ExternalInput").ap()
    gt = nc.dram_tensor("gt", [P, HW], f32, kind="ExternalInput").ap()
    mask = nc.dram_tensor("mask", [P, HW], f32, kind="ExternalInput").ap()
    out = nc.dram_tensor("out", [P, 2 * NT], f32, kind="ExternalOutput").ap()

    with TileContext(nc) as tc:
        with (
            tc.tile_pool(name="io", bufs=IO_BUFS) as io,
            tc.tile_pool(name="work", bufs=WORK_BUFS) as work,
            tc.tile_pool(name="acc", bufs=1) as accp,
        ):
            # l1 partial in column 2i, mask count in column 2i+1
            acc = accp.tile([P, 2 * NT], f32, tag="acc")

            o = 0
            for i, t in enumerate(TILE_SIZES):
                tp = io.tile([P, T], f32, tag="pre", name="tp")[:, :t]
                tg = io.tile([P, T], f32, tag="gt", name="tg")[:, :t]
                tm = io.tile([P, T], f32, tag="mask", name="tm")[:, :t]
                # mask last: its DMA completes last, and the only work
                # still gated on it (count accum + stt) is short.
                nc.sync.dma_start(out=tp, in_=pre[:, o : o + t])
                nc.sync.dma_start(out=tg, in_=gt[:, o : o + t])
                nc.sync.dma_start(out=tm, in_=mask[:, o : o + t])

                sd = work.tile([P, T], f32, tag="sd", name="sd")[:, :t]
                ad = work.tile([P, T], f32, tag="ad", name="ad")[:, :t]

                nc.vector.tensor_tensor(out=sd, in0=tp, in1=tg, op=Alu.subtract)
                # count: mask is 0/1, so sum == nonzero count. Runs on ACT or
                # Pool, parallel to the DVE chain (emitted before the abs so
                # it never blocks it); ACT's junk full-size output goes onto
                # tg (dead after the subtract).
                if COUNT_MODES[i] == "act":
                    nc.scalar.activation(
                        out=tg,
                        in_=tm,
                        func=Act.Copy,
                        accum_out=acc[:, 2 * i + 1 : 2 * i + 2],
                    )
                else:
                    # Pool stt: (tm bypass 0) * tm = tm (mask is 0/1),
                    # accum_out = sum(tm) = count
                    nc.gpsimd.scalar_tensor_tensor(
                        out=tg,
                        in0=tm,
                        scalar=0.0,
                        in1=tm,
                        op0=Alu.bypass,
                        op1=Alu.mult,
                        accum_out=acc[:, 2 * i + 1 : 2 * i + 2],
                    )
                nc.vector.tensor_tensor(out=ad, in0=sd, in1=tm, op=Alu.mult)
                if TILE_MODES[i] == "dve":
                    # |ad| + row-sum in one DVE pass: (ad * -1) max ad = |ad|
                    nc.vector.scalar_tensor_tensor(
                        out=sd,
                        in0=ad,
                        scalar=-1.0,
                        in1=ad,
                        op0=Alu.mult,
                        op1=Alu.max,
                        accum_out=acc[:, 2 * i : 2 * i + 1],
                    )
                else:
                    # |ad| + row-sum on ACT instead (keeps DVE 2-pass)
                    nc.scalar.activation(
                        out=sd,
                        in_=ad,
                        func=Act.Abs,
                        accum_out=acc[:, 2 * i : 2 * i + 1],
                    )
                o += t

            nc.sync.dma_start(out=out, in_=acc)

    nc.compile()
    _CACHE[key] = nc
    return nc


def _shard(pre, gt, mask):
    in_maps = []
    for i in range(N_CORES):
        sl = slice(i * BPC, (i + 1) * BPC)
        in_maps.append(
            {
                "pre": np.ascontiguousarray(pre[sl], dtype=np.float32).reshape(P, HW),
                "gt": np.ascontiguousarray(gt[sl], dtype=np.float32).reshape(P, HW),
                "mask": np.ascontiguousarray(mask[sl], dtype=np.float32).reshape(P, HW),
            }
        )
    return in_maps


def _combine(results, batch_size):
    total = np.float32(0.0)
    for r in results:
        o = np.asarray(r["out"], dtype=np.float32)
        l1 = o[:, 0::2].sum(axis=1, dtype=np.float32)
        ct = o[:, 1::2].sum(axis=1, dtype=np.float32)
        total += (l1 / np.maximum(ct, np.float32(1.0))).sum(dtype=np.float32)
    return np.asarray(total / np.float32(int(batch_size)), dtype=np.float32)


def run(pre, gt, mask, batch_size, trace=False, **bass_kwargs):
    from concourse.bass_utils import run_bass_kernel_spmd

    nc = _build()
    in_maps = _shard(np.asarray(pre), np.asarray(gt), np.asarray(mask))
    res = run_bass_kernel_spmd(
        nc, in_maps, list(range(N_CORES)), trace=trace, **bass_kwargs
    )
    loss = _combine(res.results, batch_size)
    return loss, res


def kernel(pre, gt, mask, batch_size):
    loss, _ = run(pre, gt, mask, batch_size)
    return loss


# revision 26
# speedup vs baseline: 1.0477x; 1.0059x over previous
"""Masked L1 loss (per-(b,c) normalized) on 8 Trainium2 NeuronCores.

Layout: batch-dim data parallel. Core i takes batches [2i, 2i+2) of the
[16, 64, 128, 128] inputs -> a [128, 16384] shard (partition = (b, c) pair,
free = h*w). Per [128, t] tile (t tapers 2048 -> 256 at the end to shorten
the post-last-DMA tail):
    DVE  tensor_tensor          sd = pre - gt
    DVE  tensor_tensor          dm = sd * mask
    ACT  activation(Copy)+accum ct_col = sum(mask)   (mask is 0/1)
    DVE  scalar_tensor_tensor   junk = (dm * -1) max dm = |dm|,
                                accum l1_col = sum   (all ISA-valid ALU ops;
                                mask is 0/1 so |dm| = |pre-gt|*mask)
DMA-bound by design (~24 MiB/core HBM reads; DVE 3 passes, ACT 1 pass,
both under the DMA stream time, so they hide; the whole l1 chain stays on
DVE so the post-last-DMA tail has no cross-engine hops).
Per-(b,c) tile partials land in one [128, 2*NT] accumulator (l1 in even
columns, count in odd), DMA'd to DRAM once at the end.
Host: l1 = even.sum, ct = odd.sum, loss = sum(l1/max(ct,1))/B.
"""

import sys

if "/opt/trn_rl_repo" not in sys.path:
    sys.path.insert(0, "/opt/trn_rl_repo")

import numpy as np

B, C, H, W = 16, 64, 128, 128
N_CORES = 8
BPC = B // N_CORES          # batches per core = 2
P = BPC * C                 # partitions per core = 128 (one (b,c) pair each)
HW = H * W                  # 16384 free elements per partition
T = 2048                    # max free-dim tile size
# Taper the end of the schedule: the last tiles are small so the critical
# path after the final input DMA (sem + stt + out-DMA) is short.
TILE_SIZES = [2048] * 7 + [1024, 704, 320]
# per-tile l1-chain placement: "dve" = |dm| sum via DVE stt (3rd DVE pass),
# "act" = |dm| sum via ACT activation(Abs)+accum (2 DVE passes + 1 ACT pass).
# Body tiles on ACT keep DVE at 2 passes/tile (no backlog at the taper);
# the last tile's chain stays entirely on DVE so the tail has no engine hop.
TILE_MODES = ["custom"] * 10
# count placement is ACT everywhere: the Pool engine's ISA rejects
# TENSOR_SCALAR_PTR (walrus neuron_isa_check_opcode_on_engine), so the
# count cannot ride gpsimd
COUNT_MODES = ["act"] * 10
IO_BUFS = 3
WORK_BUFS = 3
assert sum(TILE_SIZES) == HW
NT = len(TILE_SIZES)

_CACHE = {}


def _register_abs_mul_reduce():
    """Register the ABS_MUL_REDUCE_ANT custom DVE op (once per process):
    out = |Src0 * Src1|, accum_out = sum(out). One DVE pass for the whole
    |sd|*mask + row-sum step; the uop program ships in the per-NEFF DVE
    table like the stock custom ops. abs is composed as maxx(x, -x), the
    v3-supported form."""
    import numpy as np

    import concourse.dve_ops as dops
    from concourse.dve_ops import _CUSTOM_DVE_ROW_BASE, DveOp
    from concourse.dve_spec import AluOp, Spec, Src0, Src1, Zero, maxx

    for op in dops.OPS:
        if op.name == "ABS_MUL_REDUCE_ANT":
            return op

    def _ref_abs_mul_reduce(in0, in1, s0, s1, imm2):
        b = np.abs(in0.astype(np.float32) * in1).astype(np.float32)
        return b, b.reshape(b.shape[0], -1).sum(axis=-1, keepdims=True)

    x = Src0 * Src1
    op = DveOp(
        "ABS_MUL_REDUCE_ANT",
        Spec(
            body=maxx(x, -x),
            accum=AluOp.ADD,
            accum_init=Zero,
            reference=_ref_abs_mul_reduce,
        ),
        subdim=False,
        uops_sha={"v3": "50d5ba63e053883c", "v4": "4d7f04058b420bd7"},
    )
    dops.OPS.append(op)
    dops.CUSTOM_DVE_SPECS[op.name] = op.spec
    dops._SUB_OPCODE_FOR_NAME[op.name] = _CUSTOM_DVE_ROW_BASE + len(dops.OPS) - 1
    return op


def _build():
    key = ("nc", tuple(TILE_SIZES), tuple(TILE_MODES), tuple(COUNT_MODES),
           T, IO_BUFS, WORK_BUFS)
    if key in _CACHE:
        return _CACHE[key]

    import concourse.bacc as bacc
    import concourse.mybir as mybir
    from concourse.tile import TileContext

    f32 = mybir.dt.float32
    Alu = mybir.AluOpType
    Act = mybir.ActivationFunctionType
    absmul = _register_abs_mul_reduce()

    nc = bacc.Bacc(
        "TRN2",
        target_bir_lowering=False,
        debug=False,
        enable_asserts=False,
        num_devices=N_CORES,
    )

    pre = nc.dram_tensor("pre", [P, HW], f32, kind="ExternalInput").ap()
    gt = nc.dram_tensor("gt", [P, HW], f32, kind="ExternalInput").ap()
    mask = nc.dram_tensor("mask", [P, HW], f32, kind="ExternalInput").ap()
    out = nc.dram_tensor("out", [P, 2 * NT], f32, kind="ExternalOutput").ap()

    with TileContext(nc) as tc:
        with (
            tc.tile_pool(name="io", bufs=IO_BUFS) as io,
            tc.tile_pool(name="work", bufs=WORK_BUFS) as work,
            tc.tile_pool(name="acc", bufs=1) as accp,
        ):
            # l1 partial in column 2i, mask count in column 2i+1
            acc = accp.tile([P, 2 * NT], f32, tag="acc")

            o = 0
            for i, t in enumerate(TILE_SIZES):
                tp = io.tile([P, T], f32, tag="pre", name="tp")[:, :t]
                tg = io.tile([P, T], f32, tag="gt", name="tg")[:, :t]
                tm = io.tile([P, T], f32, tag="mask", name="tm")[:, :t]
                # mask last: its DMA completes last, and the only work
                # still gated on it (count accum + stt) is short.
                nc.sync.dma_start(out=tp, in_=pre[:, o : o + t])
                nc.sync.dma_start(out=tg, in_=gt[:, o : o + t])
                nc.sync.dma_start(out=tm, in_=mask[:, o : o + t])

                sd = work.tile([P, T], f32, tag="sd", name="sd")[:, :t]
                ad = work.tile([P, T], f32, tag="ad", name="ad")[:, :t]

                nc.vector.tensor_tensor(out=sd, in0=tp, in1=tg, op=Alu.subtract)
                # count: mask is 0/1, so sum == nonzero count. Runs on ACT or
                # Pool, parallel to the DVE chain (emitted before the abs so
                # it never blocks it); ACT's junk full-size output goes onto
                # tg (dead after the subtract).
                if COUNT_MODES[i] == "act":
                    nc.scalar.activation(
                        out=tg,
                        in_=tm,
                        func=Act.Copy,
                        accum_out=acc[:, 2 * i + 1 : 2 * i + 2],
                    )
                else:
                    # Pool stt: (tm bypass 0) * tm = tm (mask is 0/1),
                    # accum_out = sum(tm) = count
                    nc.gpsimd.scalar_tensor_tensor(
                        out=tg,
                        in0=tm,
                        scalar=0.0,
                        in1=tm,
                        op0=Alu.bypass,
                        op1=Alu.mult,
                        accum_out=acc[:, 2 * i + 1 : 2 * i + 2],
                    )
                if TILE_MODES[i] == "custom":
                    # one DVE pass: |sd * mask| + row-sum via the custom op
                    nc.vector._custom_dve(
                        absmul,
                        out=ad,
                        in0=sd,
                        in1=tm,
                        accum_out=acc[:, 2 * i : 2 * i + 1],
                    )
                elif TILE_MODES[i] == "dve":
                    nc.vector.tensor_tensor(out=ad, in0=sd, in1=tm, op=Alu.mult)
                    # |ad| + row-sum in one DVE pass: (ad * -1) max ad = |ad|
                    nc.vector.scalar_tensor_tensor(
                        out=sd,
                        in0=ad,
                        scalar=-1.0,
                        in1=ad,
                        op0=Alu.mult,
                        op1=Alu.max,
                        accum_out=acc[:, 2 * i : 2 * i + 1],
                    )
                else:
                    nc.vector.tensor_tensor(out=ad, in0=sd, in1=tm, op=Alu.mult)
                    # |ad| + row-sum on ACT instead (keeps DVE 2-pass)
                    nc.scalar.activation(
                        out=sd,
                        in_=ad,
                        func=Act.Abs,
                        accum_out=acc[:, 2 * i : 2 * i + 1],
                    )
                o += t

            nc.sync.dma_start(out=out, in_=acc)

    nc.compile()
    _CACHE[key] = nc
    return nc


def _shard(pre, gt, mask):
    in_maps = []
    for i in range(N_CORES):
        sl = slice(i * BPC, (i + 1) * BPC)
        in_maps.append(
            {
                "pre": np.ascontiguousarray(pre[sl], dtype=np.float32).reshape(P, HW),
                "gt": np.ascontiguousarray(gt[sl], dtype=np.float32).reshape(P, HW),
                "mask": np.ascontiguousarray(mask[sl], dtype=np.float32).reshape(P, HW),
            }
        )
    return in_maps


def _combine(results, batch_size):
    total = np.float32(0.0)
    for r in results:
        o = np.asarray(r["out"], dtype=np.float32)
        l1 = o[:, 0::2].sum(axis=1, dtype=np.float32)
        ct = o[:, 1::2].sum(axis=1, dtype=np.float32)
        total += (l1 / np.maximum(ct, np.float32(1.0))).sum(dtype=np.float32)
    return np.asarray(total / np.float32(int(batch_size)), dtype=np.float32)


def run(pre, gt, mask, batch_size, trace=False, **bass_kwargs):
    from concourse.bass_utils import run_bass_kernel_spmd

    nc = _build()
    in_maps = _shard(np.asarray(pre), np.asarray(gt), np.asarray(mask))
    res = run_bass_kernel_spmd(
        nc, in_maps, list(range(N_CORES)), trace=trace, **bass_kwargs
    )
    loss = _combine(res.results, batch_size)
    return loss, res


def kernel(pre, gt, mask, batch_size):
    loss, _ = run(pre, gt, mask, batch_size)
    return loss


# revision 28
# speedup vs baseline: 1.0481x; 1.0004x over previous
"""Masked L1 loss (per-(b,c) normalized) on 8 Trainium2 NeuronCores.

Layout: batch-dim data parallel. Core i takes batches [2i, 2i+2) of the
[16, 64, 128, 128] inputs -> a [128, 16384] shard (partition = (b, c) pair,
free = h*w). Per [128, t] tile (t tapers 2048 -> 256 at the end to shorten
the post-last-DMA tail):
    DVE  tensor_tensor           sd = pre - gt
    ACT  activation(Copy)+accum  ct_col = sum(mask)   (mask is 0/1)
    DVE  ABS_MUL_REDUCE_ANT      junk = |sd * mask|, accum l1_col = sum
                                 (custom DVE op registered below: one pass
                                 fuses mult+abs+row-sum; mask is 0/1 so
                                 |sd*mask| = |pre-gt|*mask)
DMA-bound by design (~24 MiB/core HBM reads; DVE 2 passes + ACT 1 pass sit
well under the DMA stream time, so they hide; the whole l1 chain stays on
DVE so the post-last-DMA tail has no cross-engine hops).
Per-(b,c) tile partials land in one [128, 2*NT] accumulator (l1 in even
columns, count in odd), DMA'd to DRAM once at the end.
Host: l1 = even.sum, ct = odd.sum, loss = sum(l1/max(ct,1))/B.
"""

import sys

if "/opt/trn_rl_repo" not in sys.path:
    sys.path.insert(0, "/opt/trn_rl_repo")

import numpy as np

B, C, H, W = 16, 64, 128, 128
N_CORES = 8
BPC = B // N_CORES          # batches per core = 2
P = BPC * C                 # partitions per core = 128 (one (b,c) pair each)
HW = H * W                  # 16384 free elements per partition
T = 2048                    # max free-dim tile size
# Taper the end of the schedule: the last tiles are small so the critical
# path after the final input DMA (sem + stt + out-DMA) is short.
TILE_SIZES = [2048] * 7 + [1280, 512, 256]
# per-tile l1-chain placement: "custom" = fused |sd*mask|+sum in one DVE
# pass (ABS_MUL_REDUCE_ANT); "dve"/"act" are the 2-instruction fallbacks
# (kept for reference/debugging)
TILE_MODES = ["custom"] * 10
# count placement is ACT everywhere: the Pool engine's ISA rejects
# TENSOR_SCALAR_PTR (walrus neuron_isa_check_opcode_on_engine), so the
# count cannot ride gpsimd
COUNT_MODES = ["act"] * 10
IO_BUFS = 3
WORK_BUFS = 3
assert sum(TILE_SIZES) == HW
NT = len(TILE_SIZES)

_CACHE = {}


def _register_abs_mul_reduce():
    """Register the ABS_MUL_REDUCE_ANT custom DVE op (once per process):
    out = |Src0 * Src1|, accum_out = sum(out). One DVE pass for the whole
    |sd|*mask + row-sum step; the uop program ships in the per-NEFF DVE
    table like the stock custom ops. abs is composed as maxx(x, -x), the
    v3-supported form."""
    import numpy as np

    import concourse.dve_ops as dops
    from concourse.dve_ops import _CUSTOM_DVE_ROW_BASE, DveOp
    from concourse.dve_spec import AluOp, Spec, Src0, Src1, Zero, maxx

    for op in dops.OPS:
        if op.name == "ABS_MUL_REDUCE_ANT":
            return op

    def _ref_abs_mul_reduce(in0, in1, s0, s1, imm2):
        b = np.abs(in0.astype(np.float32) * in1).astype(np.float32)
        return b, b.reshape(b.shape[0], -1).sum(axis=-1, keepdims=True)

    x = Src0 * Src1
    op = DveOp(
        "ABS_MUL_REDUCE_ANT",
        Spec(
            body=maxx(x, -x),
            accum=AluOp.ADD,
            accum_init=Zero,
            reference=_ref_abs_mul_reduce,
        ),
        subdim=False,
        uops_sha={"v3": "50d5ba63e053883c", "v4": "4d7f04058b420bd7"},
    )
    dops.OPS.append(op)
    dops.CUSTOM_DVE_SPECS[op.name] = op.spec
    dops._SUB_OPCODE_FOR_NAME[op.name] = _CUSTOM_DVE_ROW_BASE + len(dops.OPS) - 1
    return op


def _build():
    key = ("nc", tuple(TILE_SIZES), tuple(TILE_MODES), tuple(COUNT_MODES),
           T, IO_BUFS, WORK_BUFS)
    if key in _CACHE:
        return _CACHE[key]

    import concourse.bacc as bacc
    import concourse.mybir as mybir
    from concourse.tile import TileContext

    f32 = mybir.dt.float32
    Alu = mybir.AluOpType
    Act = mybir.ActivationFunctionType
    absmul = _register_abs_mul_reduce()

    nc = bacc.Bacc(
        "TRN2",
        target_bir_lowering=False,
        debug=False,
        enable_asserts=False,
        num_devices=N_CORES,
    )

    pre = nc.dram_tensor("pre", [P, HW], f32, kind="ExternalInput").ap()
    gt = nc.dram_tensor("gt", [P, HW], f32, kind="ExternalInput").ap()
    mask = nc.dram_tensor("mask", [P, HW], f32, kind="ExternalInput").ap()
    out = nc.dram_tensor("out", [P, 2 * NT], f32, kind="ExternalOutput").ap()

    with TileContext(nc) as tc:
        with (
            tc.tile_pool(name="io", bufs=IO_BUFS) as io,
            tc.tile_pool(name="work", bufs=WORK_BUFS) as work,
            tc.tile_pool(name="acc", bufs=1) as accp,
        ):
            # l1 partial in column 2i, mask count in column 2i+1
            acc = accp.tile([P, 2 * NT], f32, tag="acc")

            o = 0
            for i, t in enumerate(TILE_SIZES):
                tp = io.tile([P, T], f32, tag="pre", name="tp")[:, :t]
                tg = io.tile([P, T], f32, tag="gt", name="tg")[:, :t]
                tm = io.tile([P, T], f32, tag="mask", name="tm")[:, :t]
                # mask last: its DMA completes last, and the only work
                # still gated on it (count accum + stt) is short.
                nc.sync.dma_start(out=tp, in_=pre[:, o : o + t])
                nc.sync.dma_start(out=tg, in_=gt[:, o : o + t])
                nc.sync.dma_start(out=tm, in_=mask[:, o : o + t])

                sd = work.tile([P, T], f32, tag="sd", name="sd")[:, :t]
                ad = work.tile([P, T], f32, tag="ad", name="ad")[:, :t]

                nc.vector.tensor_tensor(out=sd, in0=tp, in1=tg, op=Alu.subtract)
                # count: mask is 0/1, so sum == nonzero count. Runs on ACT or
                # Pool, parallel to the DVE chain (emitted before the abs so
                # it never blocks it); ACT's junk full-size output goes onto
                # tg (dead after the subtract).
                if COUNT_MODES[i] == "act":
                    nc.scalar.activation(
                        out=tg,
                        in_=tm,
                        func=Act.Copy,
                        accum_out=acc[:, 2 * i + 1 : 2 * i + 2],
                    )
                else:
                    # Pool stt: (tm bypass 0) * tm = tm (mask is 0/1),
                    # accum_out = sum(tm) = count
                    nc.gpsimd.scalar_tensor_tensor(
                        out=tg,
                        in0=tm,
                        scalar=0.0,
                        in1=tm,
                        op0=Alu.bypass,
                        op1=Alu.mult,
                        accum_out=acc[:, 2 * i + 1 : 2 * i + 2],
                    )
                if TILE_MODES[i] == "custom":
                    # one DVE pass: |sd * mask| + row-sum via the custom op
                    nc.vector._custom_dve(
                        absmul,
                        out=ad,
                        in0=sd,
                        in1=tm,
                        accum_out=acc[:, 2 * i : 2 * i + 1],
                    )
                elif TILE_MODES[i] == "dve":
                    nc.vector.tensor_tensor(out=ad, in0=sd, in1=tm, op=Alu.mult)
                    # |ad| + row-sum in one DVE pass: (ad * -1) max ad = |ad|
                    nc.vector.scalar_tensor_tensor(
                        out=sd,
                        in0=ad,
                        scalar=-1.0,
                        in1=ad,
                        op0=Alu.mult,
                        op1=Alu.max,
                        accum_out=acc[:, 2 * i : 2 * i + 1],
                    )
                else:
                    nc.vector.tensor_tensor(out=ad, in0=sd, in1=tm, op=Alu.mult)
                    # |ad| + row-sum on ACT instead (keeps DVE 2-pass)
                    nc.scalar.activation(
                        out=sd,
                        in_=ad,
                        func=Act.Abs,
                        accum_out=acc[:, 2 * i : 2 * i + 1],
                    )
                o += t

            nc.sync.dma_start(out=out, in_=acc)

    nc.compile()
    _CACHE[key] = nc
    return nc


def _shard(pre, gt, mask):
    in_maps = []
    for i in range(N_CORES):
        sl = slice(i * BPC, (i + 1) * BPC)
        in_maps.append(
            {
                "pre": np.ascontiguousarray(pre[sl], dtype=np.float32).reshape(P, HW),
                "gt": np.ascontiguousarray(gt[sl], dtype=np.float32).reshape(P, HW),
                "mask": np.ascontiguousarray(mask[sl], dtype=np.float32).reshape(P, HW),
            }
        )
    return in_maps


def _combine(results, batch_size):
    total = np.float32(0.0)
    for r in results:
        o = np.asarray(r["out"], dtype=np.float32)
        l1 = o[:, 0::2].sum(axis=1, dtype=np.float32)
        ct = o[:, 1::2].sum(axis=1, dtype=np.float32)
        total += (l1 / np.maximum(ct, np.float32(1.0))).sum(dtype=np.float32)
    return np.asarray(total / np.float32(int(batch_size)), dtype=np.float32)


def run(pre, gt, mask, batch_size, trace=False, **bass_kwargs):
    from concourse.bass_utils import run_bass_kernel_spmd

    nc = _build()
    in_maps = _shard(np.asarray(pre), np.asarray(gt), np.asarray(mask))
    res = run_bass_kernel_spmd(
        nc, in_maps, list(range(N_CORES)), trace=trace, **bass_kwargs
    )
    loss = _combine(res.results, batch_size)
    return loss, res


def kernel(pre, gt, mask, batch_size):
    loss, _ = run(pre, gt, mask, batch_size)
    return loss
